# revision 1
# baseline (speedup 1.0000x reference)
"""Butterfly layer kernel for 8 Trainium2 NeuronCores.

Pure data parallelism: batch dim B=1024 is sharded 8 ways (128 per core);
all butterfly filters (<1 MB total) are replicated to every core. Each core
runs the full butterfly tree (input conv -> 10 branching conv levels ->
per-branch dense) on its batch shard; results are concatenated on the host.
"""

import numpy as np

# Hardcoded problem shape (nn_ButterflyLayer, spec.json)
B = 1024
IN_SIZ = 16384
OUT_SIZ = 16384
C = 8
NLVL = 10
IFS = 16   # IN_SIZ // 2**NLVL
KLVL = 10
OFS = 16   # OUT_SIZ // 2**KLVL
N_CORES = 8
B_LOC = B // N_CORES  # 128


def _butterfly_np(x, in_filter, in_bias, Ws, bs, fea_dense, xp):
    """Full butterfly forward for one batch shard using module `xp`
    (numpy or jax.numpy). x: [b, IN_SIZ, 1] -> out [b, OUT_SIZ, 1]."""
    b = x.shape[0]
    xin = x[..., 0].reshape(b, 2**NLVL, IFS)
    v = xp.einsum('bnf,fc->bnc', xin, in_filter[:, 0, :]) + in_bias
    v = xp.maximum(v, 0.0)
    v = v[None]  # [1, b, 1024, C]
    for lvl in range(NLVL):
        W, bias = Ws[lvl], bs[lvl]
        Kp, Bn, L, Cc = v.shape
        xpar = v.reshape(Kp, Bn, L // 2, 2, Cc)
        xr = xp.repeat(xpar, 2, axis=0)
        y = xp.einsum('kbtsc,kscd->kbtd', xr, W) + bias[:, None, None, :]
        v = xp.maximum(y, 0.0)
    out = xp.einsum('kbc,kcf->bkf', v[:, :, 0, :], fea_dense)
    return out.reshape(b, OUT_SIZ, 1)


def _run_sharded_jax(x, in_filter, in_bias, Ws, bs, fea_dense):
    """Run batch-sharded across the 8 NeuronCores via jax.pmap."""
    import jax
    import jax.numpy as jnp

    devs = jax.devices()[:N_CORES]
    assert len(devs) == N_CORES, f"need {N_CORES} cores, got {len(devs)}"

    def shard_fn(xs, in_filter, in_bias, Wflat, fea_dense):
        Ws_l = [Wflat[2 * i] for i in range(NLVL)]
        bs_l = [Wflat[2 * i + 1] for i in range(NLVL)]
        return _butterfly_np(xs, in_filter, in_bias, Ws_l, bs_l, fea_dense, jnp)

    Wflat = []
    for i in range(NLVL):
        Wflat.append(Ws[i])
        Wflat.append(bs[i])

    pf = jax.pmap(
        shard_fn,
        axis_name='i',
        in_axes=(0, None, None, None, None),
        devices=devs,
    )
    xs = x.reshape(N_CORES, B_LOC, IN_SIZ, 1)
    out = pf(xs, in_filter, in_bias, Wflat, fea_dense)
    out = np.asarray(out)  # [8, 128, OUT_SIZ, 1]
    return out.reshape(B, OUT_SIZ, 1)


def kernel(**inputs):
    x = np.asarray(inputs["x"], dtype=np.float32)
    in_filter = np.asarray(inputs["in_filter"], dtype=np.float32)
    in_bias = np.asarray(inputs["in_bias"], dtype=np.float32)
    Ws = [np.asarray(inputs[f"W{l}"], dtype=np.float32) for l in range(1, NLVL + 1)]
    bs = [np.asarray(inputs[f"b{l}"], dtype=np.float32) for l in range(1, NLVL + 1)]
    fea_dense = np.asarray(inputs["fea_dense"], dtype=np.float32)

    try:
        out = _run_sharded_jax(x, in_filter, in_bias, Ws, bs, fea_dense)
    except Exception:
        # Fallback: compute shard-by-shard on host so the kernel always
        # returns the correct full-shape output.
        outs = []
        for i in range(N_CORES):
            sh = x[i * B_LOC:(i + 1) * B_LOC]
            outs.append(
                _butterfly_np(sh, in_filter, in_bias, Ws, bs, fea_dense, np)
            )
        out = np.concatenate(outs, axis=0)
    return out.astype(np.float32)


if __name__ == "__main__":
    rng = np.random.default_rng(0)
    fake = {
        "x": rng.standard_normal((B, IN_SIZ, 1), dtype=np.float32),
        "in_filter": rng.standard_normal((IFS, 1, C), dtype=np.float32),
        "in_bias": np.zeros((C,), np.float32),
        "fea_dense": rng.standard_normal((2**KLVL, C, OFS), dtype=np.float32),
    }
    for l in range(1, NLVL + 1):
        fake[f"W{l}"] = rng.standard_normal((2**l, 2, C, C), dtype=np.float32)
        fake[f"b{l}"] = np.zeros((2**l, C), np.float32)
    out = kernel(**fake)
    print(out.shape, out.dtype)


# revision 2
# speedup vs baseline: 1.0897x; 1.0897x over previous
"""Butterfly layer kernel for 8 Trainium2 NeuronCores.

Pure data parallelism: batch dim B=1024 is sharded 8 ways (128 per core);
all butterfly filters (<1 MB total) are replicated to every core. Each core
runs the full butterfly tree (input conv -> 10 branching conv levels ->
per-branch dense) on its batch shard; results are concatenated on the host.
"""

import numpy as np

# Hardcoded problem shape (nn_ButterflyLayer, spec.json)
B = 1024
IN_SIZ = 16384
OUT_SIZ = 16384
C = 8
NLVL = 10
IFS = 16   # IN_SIZ // 2**NLVL
KLVL = 10
OFS = 16   # OUT_SIZ // 2**KLVL
N_CORES = 8
B_LOC = B // N_CORES  # 128


def _butterfly_np(x, in_filter, in_bias, Ws, bs, fea_dense, xp):
    """Full butterfly forward for one batch shard using module `xp`
    (numpy or jax.numpy). x: [b, IN_SIZ, 1] -> out [b, OUT_SIZ, 1]."""
    b = x.shape[0]
    xin = x[..., 0].reshape(b, 2**NLVL, IFS)
    v = xp.einsum('bnf,fc->bnc', xin, in_filter[:, 0, :]) + in_bias
    v = xp.maximum(v, 0.0)
    v = v[None]  # [1, b, 1024, C]
    for lvl in range(NLVL):
        W, bias = Ws[lvl], bs[lvl]
        Kp, Bn, L, Cc = v.shape
        xpar = v.reshape(Kp, Bn, L // 2, 2, Cc)
        xr = xp.repeat(xpar, 2, axis=0)
        y = xp.einsum('kbtsc,kscd->kbtd', xr, W) + bias[:, None, None, :]
        v = xp.maximum(y, 0.0)
    out = xp.einsum('kbc,kcf->bkf', v[:, :, 0, :], fea_dense)
    return out.reshape(b, OUT_SIZ, 1)


_PF_CACHE = {}


def _get_pmap():
    """Build (once) the pmap callable over the 8 NeuronCores."""
    if "pf" in _PF_CACHE:
        return _PF_CACHE["pf"]
    import jax
    import jax.numpy as jnp

    devs = jax.devices()[:N_CORES]
    assert len(devs) == N_CORES, f"need {N_CORES} cores, got {len(devs)}"

    def shard_fn(xs, in_filter, in_bias, Wflat, fea_dense):
        Ws_l = [Wflat[2 * i] for i in range(NLVL)]
        bs_l = [Wflat[2 * i + 1] for i in range(NLVL)]
        return _butterfly_np(xs, in_filter, in_bias, Ws_l, bs_l, fea_dense, jnp)

    pf = jax.pmap(
        shard_fn,
        axis_name='i',
        in_axes=(0, None, None, None, None),
        devices=devs,
    )
    _PF_CACHE["pf"] = pf
    return pf


def _run_sharded_jax(x, in_filter, in_bias, Ws, bs, fea_dense):
    """Run batch-sharded across the 8 NeuronCores via jax.pmap."""
    Wflat = []
    for i in range(NLVL):
        Wflat.append(Ws[i])
        Wflat.append(bs[i])

    pf = _get_pmap()
    xs = x.reshape(N_CORES, B_LOC, IN_SIZ, 1)
    out = pf(xs, in_filter, in_bias, Wflat, fea_dense)
    out = np.asarray(out)  # [8, 128, OUT_SIZ, 1]
    return out.reshape(B, OUT_SIZ, 1)


def kernel(**inputs):
    x = np.asarray(inputs["x"], dtype=np.float32)
    in_filter = np.asarray(inputs["in_filter"], dtype=np.float32)
    in_bias = np.asarray(inputs["in_bias"], dtype=np.float32)
    Ws = [np.asarray(inputs[f"W{l}"], dtype=np.float32) for l in range(1, NLVL + 1)]
    bs = [np.asarray(inputs[f"b{l}"], dtype=np.float32) for l in range(1, NLVL + 1)]
    fea_dense = np.asarray(inputs["fea_dense"], dtype=np.float32)

    try:
        out = _run_sharded_jax(x, in_filter, in_bias, Ws, bs, fea_dense)
    except Exception:
        # Fallback: compute shard-by-shard on host so the kernel always
        # returns the correct full-shape output.
        outs = []
        for i in range(N_CORES):
            sh = x[i * B_LOC:(i + 1) * B_LOC]
            outs.append(
                _butterfly_np(sh, in_filter, in_bias, Ws, bs, fea_dense, np)
            )
        out = np.concatenate(outs, axis=0)
    return out.astype(np.float32)


if __name__ == "__main__":
    rng = np.random.default_rng(0)
    fake = {
        "x": rng.standard_normal((B, IN_SIZ, 1), dtype=np.float32),
        "in_filter": rng.standard_normal((IFS, 1, C), dtype=np.float32),
        "in_bias": np.zeros((C,), np.float32),
        "fea_dense": rng.standard_normal((2**KLVL, C, OFS), dtype=np.float32),
    }
    for l in range(1, NLVL + 1):
        fake[f"W{l}"] = rng.standard_normal((2**l, 2, C, C), dtype=np.float32)
        fake[f"b{l}"] = np.zeros((2**l, C), np.float32)
    out = kernel(**fake)
    print(out.shape, out.dtype)
